# revision 2
# baseline (speedup 1.0000x reference)
"""CenterLoss kernel for Trainium2, 8-core SPMD.

loss = mean_i( 0.5 * || x[i] - centers[labels[i]] ||^2 )

Sharding: data-parallel over the batch. Each of the 8 cores gets 128
samples (x rows + labels) and a replicated view of the full centers
table in its HBM. On-device, each core indirect-DMA-gathers only the
128 label-matched center rows (256 KB) — never touching the other
64 MB of the table — computes the fused squared-distance row-reduce on
the vector engine, and reduces across partitions with a ones-vector
matmul on the tensor engine. Each core emits one partial sum; the
host combines 8 scalars (the unshard step) and applies the 0.5/N
scaling.
"""

import numpy as np

N_CORES = 8
BATCH = 1024
FEAT = 512
NUM_CLASSES = 32768
B_LOC = BATCH // N_CORES  # 128 == SBUF partition count

_compiled = None


def _build():
    import concourse.bacc as bacc
    import concourse.bass as bass
    import concourse.mybir as mybir
    import concourse.tile as tile

    f32 = mybir.dt.float32
    i32 = mybir.dt.int32

    nc = bacc.Bacc(
        "TRN2",
        target_bir_lowering=False,
        debug=False,
        enable_asserts=False,
        num_devices=N_CORES,
    )

    x_d = nc.dram_tensor("x", [B_LOC, FEAT], f32, kind="ExternalInput").ap()
    lab_d = nc.dram_tensor("labels", [B_LOC, 1], i32, kind="ExternalInput").ap()
    cen_d = nc.dram_tensor(
        "centers", [NUM_CLASSES, FEAT], f32, kind="ExternalInput"
    ).ap()
    out_d = nc.dram_tensor("out", [1, 1], f32, kind="ExternalOutput").ap()

    with tile.TileContext(nc) as tc:
        with (
            tc.tile_pool(name="sb", bufs=1) as pool,
            tc.tile_pool(name="ps", bufs=1, space="PSUM") as pp,
        ):
            xt = pool.tile([B_LOC, FEAT], dtype=f32)
            ct = pool.tile([B_LOC, FEAT], dtype=f32)
            lab = pool.tile([B_LOC, 1], dtype=i32)

            nc.sync.dma_start(out=xt[:], in_=x_d[:, :])
            nc.sync.dma_start(out=lab[:], in_=lab_d[:, :])
            nc.gpsimd.indirect_dma_start(
                out=ct[:],
                out_offset=None,
                in_=cen_d[:, :],
                in_offset=bass.IndirectOffsetOnAxis(ap=lab[:, :1], axis=0),
            )

            diff = pool.tile([B_LOC, FEAT], dtype=f32)
            nc.vector.tensor_sub(out=diff[:], in0=xt[:], in1=ct[:])

            # One ACT op: sq = diff*diff AND row = sum_free(sq)
            sq = pool.tile([B_LOC, FEAT], dtype=f32)
            row = pool.tile([B_LOC, 1], dtype=f32)
            nc.scalar.activation(
                out=sq[:],
                in_=diff[:],
                func=mybir.ActivationFunctionType.Square,
                accum_out=row[:],
            )

            # Partition reduce: ones[128,1].T @ row[128,1] -> [1,1] PSUM
            ones = pool.tile([B_LOC, 1], dtype=f32)
            nc.vector.memset(ones[:], 1.0)
            acc = pp.tile([1, 1], dtype=f32, space="PSUM")
            nc.tensor.matmul(out=acc[:], lhsT=ones[:], rhs=row[:], start=True, stop=True)

            res = pool.tile([1, 1], dtype=f32)
            nc.vector.tensor_copy(out=res[:], in_=acc[:])
            nc.sync.dma_start(out=out_d[:, :], in_=res[:])

    nc.compile()
    return nc


def _get_compiled():
    global _compiled
    if _compiled is None:
        _compiled = _build()
    return _compiled


def _in_maps(x, labels, centers):
    xs = np.ascontiguousarray(np.asarray(x, dtype=np.float32)).reshape(
        N_CORES, B_LOC, FEAT
    )
    lab32 = np.ascontiguousarray(
        np.asarray(labels).astype(np.int32).reshape(N_CORES, B_LOC, 1)
    )
    cen = np.ascontiguousarray(np.asarray(centers, dtype=np.float32))
    return [
        {"x": xs[i], "labels": lab32[i], "centers": cen} for i in range(N_CORES)
    ]


def kernel(x, labels, centers):
    from concourse.bass_utils import run_bass_kernel_spmd

    nc = _get_compiled()
    res = run_bass_kernel_spmd(nc, _in_maps(x, labels, centers), list(range(N_CORES)))
    partials = np.array(
        [np.float64(r["out"].reshape(())) for r in res.results], dtype=np.float64
    )
    total = 0.5 * partials.sum() / BATCH
    return np.asarray(total, dtype=np.float32)
